# revision 9
# baseline (speedup 1.0000x reference)
"""Trainium2 Bass kernel for the DeepSeek-V4 indexer compressor (prefill).

Contract: kernel(**inputs) takes the FULL unsharded inputs (numpy) and
returns the FULL [1, 2048, 128] float32 output.

Strategy (8 NeuronCores, sequence-parallel):
  - Each core handles 1024 tokens = 256 compress blocks.  The 4-token
    halo needed by the overlap transform is folded IN-BAND: every chunk's
    rhs has 4 extra leading columns holding the 4 tokens just before the
    chunk (prev core's tokens for chunk 0; zeros+mask on core 0), so the
    halo rides the main matmul stream at full rate instead of 114 tiny
    matmuls with weight reloads.
  - On-device layout is channel-major: the contraction dim (7168) on SBUF
    partitions, tokens on the free axis.  Host pre-transposes/bf16-casts
    x; wkv/wgate fuse into one [7168, 512] matrix with channel order
    [kv_lo | sc_lo | kv_hi | sc_hi].  The intra-window pos-emb (ape) is
    added on the DVE during the epilogue (a [128, 512] per-phase tiled
    const), not via extra contraction rows, shaving 4 matmuls per chunk.
  - 1024 own tokens in 4 chunks (508, 224, 224, 68); chunk 0 unpacked
    (4 full PSUM banks, 512 cols each incl. halo), the rest packed 2-per
    bank with the m-order (0,2,1,3) alternating physical banks.  56
    k-chunks accumulate per chunk; only the first matmul per bank sets
    start (start clears has_written for the whole bank).
  - Epilogue per chunk: softmax via ACT exp (the ONLY ACT function used,
    so the activation table is loaded exactly once - no EXP/SQRT table
    thrash on the critical path) + DVE quad-reduces; comp = A/Z with a
    single DVE divide; RMSNorm is deferred past the rotary+FWHT (a
    per-block column scale commutes with both), with norm_w folded into
    the cos/sin tables, so the chain forks into two parallel branches:
      rot: t1,t2 muls -> pair-swap perm matmul -> add -> FWHT matmul
      rs:  sq -> ones-matmul varsum -> (v+eps)^-0.5 via one DVE pow ->
           row-broadcast matmul
    joined by one DVE multiply straight into the output SBUF tile.  The
    small matmuls run in bf16 (exact or <0.4% error) at full PE rate.
  - DMA: weights stream on the scalar-engine HWDGE queue, x on the sync
    queue (two hardware queues run the 16 DMA engines closer to the
    358 GB/s HBM roof than one); first pieces are split small for fast
    first arrival.  Warm-up matmuls on a scalar-memzeroed tile bridge
    the ~7 us framework preamble so the PE clock gate is ramped when
    real data lands.
  Output stays channel-major; host transposes back.
"""

import math
import os

import numpy as np
import ml_dtypes

import concourse.bass as bass
import concourse.bacc as bacc
import concourse.tile as tile
import concourse.mybir as mybir
from concourse.bass_utils import run_bass_kernel_spmd

BF16 = ml_dtypes.bfloat16
F32 = np.float32

# Problem dims (hardcoded per contract)
DIM = 7168
HD = 128
RATIO = 4
COFF = 2
SEQ = 8192
NB = SEQ // RATIO            # 2048 compressed blocks
NCORES = 8
TOK = SEQ // NCORES          # 1024 own tokens per core
NBC = TOK // RATIO           # 256 blocks per core
KC = DIM // 128              # 56 contraction chunks
G = 8                        # k-chunks per w DMA group
NG = KC // G                 # 7 groups
CHUNKS = (508, 224, 224, 68)  # own tokens per chunk; +4 in-band halo
OFFS = (0, 508, 732, 956)     # own-token offsets
BOFF = (0, 127, 183, 239)     # block offsets
NMISC = 3                    # rotating epilogue PSUM banks
EPS = 1e-6
NEGB = -300.0                # exp(x - 300) == 0.0 in fp32 for masked rows
NDUMMY = 7                   # warm-up matmuls (PE clock ramp)
NBDUMMY = 4                  # chunk0->1 gap fillers

# f32 const pack column layout
C_CD = 0             # cdup (cos * norm_w dup) [128, 256]
C_SD = 256           # sdup (signed sin * norm_w dup) [128, 256]
C_AL = 512           # ape_lo tiled by phase [128, 512]
C_AH = 1024          # ape_hi tiled by phase [128, 512]
C_HB = 1536          # halo mask bias [128, 1]
C_EP = 1537          # eps row (row 0 only) [1, 128]
C_MG = 1665          # rsqrt magic 0x5f3759df bits (row 0) [1, 128]
C_ON = 1793          # int 1 bits (row 0) [1, 128]
C_15 = 1921          # 1.5 row (row 0) [1, 128]
C_TOT = 2056

# bf16 const pack column layout
B_PM = 0             # pair-swap perm matrix [128, 128]
B_HM = 128           # FWHT matrix [128, 128]
B_R1 = 256           # ones row (row 0 only) [1, 128]
B_OK = 384           # 1/HD column [128, 1]
B_TOT = 392

_cache = {}


def _fwht_mat():
    """fwht(v) = M @ v for the reference's butterfly; fwht(I) = M.T which
    is exactly the lhsT the tensor engine wants."""
    y = np.eye(HD, dtype=np.float64)
    d = HD
    for _ in range(int(math.log2(d))):
        y = y.reshape(y.shape[:-1] + (2, -1))
        a, b = y[..., 0, :], y[..., 1, :]
        y = np.concatenate([a + b, a - b], axis=-1)
    scale = np.float32(d) ** np.float32(-0.5)
    return (y * scale).astype(F32)


def _build_nc():
    nc = bacc.Bacc("TRN2", target_bir_lowering=False)
    f32 = mybir.dt.float32
    bf16 = mybir.dt.bfloat16

    # x, packed chunk-major: rows [(c,g) x 128], cols [cc*cols + t]
    xpA_d = nc.dram_tensor("xpA", [NG * 128, G * 512], bf16, kind="ExternalInput")
    xpB_d = nc.dram_tensor("xpB", [2 * NG * 128, G * 228], bf16, kind="ExternalInput")
    xpC_d = nc.dram_tensor("xpC", [NG * 128, G * 72], bf16, kind="ExternalInput")
    wp_d = nc.dram_tensor("wp", [NG * 128, G * 512], bf16, kind="ExternalInput")
    cpk_d = nc.dram_tensor("cpk", [128, C_TOT], f32, kind="ExternalInput")
    cbk_d = nc.dram_tensor("cbk", [128, B_TOT], bf16, kind="ExternalInput")
    out_d = nc.dram_tensor("out", [128, NBC], f32, kind="ExternalOutput")

    AX = mybir.AxisListType
    OP = mybir.AluOpType
    AF = mybir.ActivationFunctionType

    with tile.TileContext(nc) as tc:
        with (
            tc.tile_pool(name="wts", bufs=1) as wts,
            tc.tile_pool(name="csts", bufs=1) as csts,
            tc.tile_pool(name="xs", bufs=11) as xs,
            tc.tile_pool(name="epi", bufs=2) as epi,
            tc.tile_pool(name="ps", bufs=2, space="PSUM") as ps,
            tc.tile_pool(name="mps", bufs=1, space="PSUM") as mps,
        ):
            # rotating PSUM banks for the epilogue's small matmul outputs:
            # [0:nloc rsb][128:+nloc perm][256:+nloc fwht][384:+nloc varsum]
            miscs = [
                mps.tile([128, 512], mybir.dt.float32, name=f"misc{i}", tag=f"misc{i}")
                for i in range(NMISC)
            ]

            cpk = csts.tile([128, C_TOT], f32, name="cpk", tag="cpk")
            nc.gpsimd.dma_start(out=cpk, in_=cpk_d[:, :])
            cbk = csts.tile([128, B_TOT], bf16, name="cbk", tag="cbk")
            nc.gpsimd.dma_start(out=cbk, in_=cbk_d[:, :])
            cdup = cpk[:, C_CD:C_CD + NBC]
            sdup = cpk[:, C_SD:C_SD + NBC]
            apeL = cpk[:, C_AL:C_AL + 512]
            apeH = cpk[:, C_AH:C_AH + 512]
            hbias = cpk[:, C_HB:C_HB + 1]
            epsrow = cpk[0:1, C_EP:C_EP + 128]
            magicrow = cpk[0:1, C_MG:C_MG + 128]
            oneirow = cpk[0:1, C_ON:C_ON + 128]
            c15row = cpk[0:1, C_15:C_15 + 128]
            pmat = cbk[:, B_PM:B_PM + 128]
            hmat = cbk[:, B_HM:B_HM + 128]
            row1 = cbk[0:1, B_R1:B_R1 + 128]
            onesk = cbk[:, B_OK:B_OK + 1]

            outsb = csts.tile([128, NBC], f32, name="outsb", tag="outsb")

            # PE warm-up on a scalar-memzeroed tile: no vector-engine dep,
            # dummies start right after the framework barrier and ramp the
            # PE clock gate while the first DMA pieces stream in.
            zt = csts.tile([128, 512], bf16, name="zt", tag="zt")
            nc.scalar.memzero(zt)
            for i in range(NDUMMY):
                nc.tensor.matmul(miscs[-1][:, :], zt[:, 0:128], zt[:, :],
                                 start=True, stop=True)

            # ---- weight loads on the scalar HWDGE queue (parallel with x
            # on the sync queue); g=0 split for fast first arrival ----
            wt = []
            for g in range(NG):
                wtg = wts.tile([128, G * 512], bf16, name=f"wt{g}", tag=f"wt{g}")
                wt.append(wtg)
            for g in range(NG):
                if g == 0:
                    for a in range(0, G * 512, 1024):
                        nc.scalar.dma_start(out=wt[0][:, a:a + 1024],
                                            in_=wp_d[0:128, a:a + 1024])
                else:
                    nc.scalar.dma_start(out=wt[g],
                                        in_=wp_d[128 * g:128 * (g + 1), :])

            # ---- x loads on the sync HWDGE queue, chunk-major ----
            def load_xq(ci, g):
                cols = CHUNKS[ci] + RATIO
                if ci == 0:
                    xq = xs.tile([128, G * 512], bf16, name=f"xq{ci}{g}",
                                 tag="xqbig", bufs=4)
                    if g == 0:
                        for a in range(0, G * 512, 1024):
                            nc.sync.dma_start(out=xq[:, a:a + 1024],
                                              in_=xpA_d[0:128, a:a + 1024])
                    else:
                        nc.sync.dma_start(
                            out=xq, in_=xpA_d[128 * g:128 * (g + 1), :])
                    return xq
                xqt = xs.tile([128, G * 228], bf16, name=f"xq{ci}{g}",
                              tag="xqs", bufs=7)
                xq = xqt[:, 0:G * cols]
                src, r0 = (xpB_d, ((ci - 1) * NG + g) * 128) if ci < 3 \
                    else (xpC_d, g * 128)
                nc.sync.dma_start(out=xq, in_=src[r0:r0 + 128, 0:G * cols])
                return xq

            xq_pre = {}

            def chunk_matmuls(ci, outs, packed):
                """56-k-chunk accumulation over CHUNKS[ci]+4 columns.  For
                packed banks, start=True clears has_written for the WHOLE
                bank, so only the first matmul per bank sets it; m-order
                (0,2,1,3) alternates physical banks between consecutive
                matmuls."""
                cols = CHUNKS[ci] + RATIO
                order = (0, 2, 1, 3) if packed else (0, 1, 2, 3)
                for g in range(NG):
                    xq = xq_pre.pop((ci, g), None)
                    if xq is None:
                        xq = load_xq(ci, g)
                    if ci + 1 < len(CHUNKS) and g == NG - 1:
                        for gg in range(NG):
                            xq_pre[(ci + 1, gg)] = load_xq(ci + 1, gg)
                    for cc in range(G):
                        first = g == 0 and cc == 0
                        last = g == NG - 1 and cc == G - 1
                        for m in order:
                            st = first and (m in (0, 2) if packed else True)
                            sp = last and (m in (1, 3) if packed else True)
                            nc.tensor.matmul(
                                outs[m],
                                wt[g][:, cc * 512 + 128 * m:cc * 512 + 128 * (m + 1)],
                                xq[:, cc * cols:(cc + 1) * cols],
                                start=st,
                                stop=sp,
                                skip_group_check=True,
                            )

            def epilogue(ci, psums):
                """Softmax+RMS+rotary+FWHT for the blocks of chunk ci."""
                Q = CHUNKS[ci]
                nloc = Q // RATIO
                b0 = BOFF[ci]
                kv1p, sc1p, kv2p, sc2p = psums
                misc = miscs[ci % NMISC]

                E = epi.tile([128, 1024], mybir.dt.float32, name=f"E{ci}", tag="E")
                M = epi.tile([128, 1024], mybir.dt.float32, name=f"M{ci}", tag="M")
                E = E[:, 0:2 * Q]
                M = M[:, 0:2 * Q]
                if ci == 0:
                    nc.scalar.activation(E[:, 0:4], sc1p[:, 0:4], AF.Exp,
                                         bias=hbias)
                    nc.scalar.activation(E[:, 4:Q], sc1p[:, 4:Q], AF.Exp)
                else:
                    nc.scalar.activation(E[:, 0:Q], sc1p[:, 0:Q], AF.Exp)
                nc.scalar.activation(E[:, Q:2 * Q], sc2p[:, 4:4 + Q], AF.Exp)
                # kv + ape (per-phase tiled const), then *= E in place
                nc.vector.tensor_tensor(M[:, 0:Q], kv1p[:, 0:Q],
                                        apeL[:, 0:Q], op=OP.add)
                nc.vector.tensor_tensor(M[:, Q:2 * Q], kv2p[:, 4:4 + Q],
                                        apeH[:, 0:Q], op=OP.add)
                nc.vector.tensor_tensor(M[:, 0:Q], M[:, 0:Q], E[:, 0:Q],
                                        op=OP.mult)
                nc.vector.tensor_tensor(M[:, Q:2 * Q], M[:, Q:2 * Q],
                                        E[:, Q:2 * Q], op=OP.mult)

                Z = epi.tile([128, 128], mybir.dt.float32, name=f"Z{ci}", tag="Z")
                Z = Z[:, 0:nloc]
                nc.vector.tensor_reduce(
                    Z, E.rearrange("p (t n q) -> p n t q", t=2, q=RATIO),
                    axis=AX.XY, op=OP.add)
                A = epi.tile([128, 128], mybir.dt.float32, name=f"A{ci}", tag="A")
                A = A[:, 0:nloc]
                nc.vector.tensor_reduce(
                    A, M.rearrange("p (t n q) -> p n t q", t=2, q=RATIO),
                    axis=AX.XY, op=OP.add)
                Zr = epi.tile([128, 128], mybir.dt.float32, name=f"Zr{ci}", tag="Zr")
                Zr = Zr[:, 0:nloc]
                nc.vector.reciprocal(Zr, Z)
                comp = epi.tile([128, 128], mybir.dt.float32, name=f"cp{ci}", tag="cp")
                comp = comp[:, 0:nloc]
                nc.vector.tensor_tensor(comp, A, Zr, op=OP.mult)

                # branch rs: sq -> varsum matmul -> (v+eps)^-0.5 -> bcast mm
                sq = epi.tile([128, 128], mybir.dt.bfloat16, name=f"sq{ci}", tag="sq")
                sq = sq[:, 0:nloc]
                nc.vector.tensor_tensor(sq, comp, comp, op=OP.mult)
                nc.tensor.matmul(misc[0:1, 384:384 + nloc], onesk, sq[:, :],
                                 start=True, stop=True)
                # rs = (v+eps)^-0.5 all-DVE: bitcast magic seed + one Newton
                # step (pow/divide are not valid DVE ALU ops; ACT Sqrt would
                # thrash the activation table against Exp)
                i32 = mybir.dt.int32
                veps = epi.tile([1, 128], mybir.dt.float32,
                                name=f"ve{ci}", tag="ve")[:, 0:nloc]
                nc.vector.tensor_tensor(veps, misc[0:1, 384:384 + nloc],
                                        epsrow[:, 0:nloc], op=OP.add)
                ii = epi.tile([1, 128], i32, name=f"ii{ci}", tag="ii")[:, 0:nloc]
                nc.vector.tensor_tensor(ii, veps.bitcast(i32),
                                        oneirow[:, 0:nloc].bitcast(i32),
                                        op=OP.arith_shift_right)
                jj = epi.tile([1, 128], i32, name=f"jj{ci}", tag="jj")[:, 0:nloc]
                nc.vector.tensor_tensor(jj, magicrow[:, 0:nloc].bitcast(i32),
                                        ii, op=OP.subtract)
                y0 = jj.bitcast(mybir.dt.float32)
                y2 = epi.tile([1, 128], mybir.dt.float32,
                              name=f"y2{ci}", tag="y2")[:, 0:nloc]
                nc.vector.tensor_tensor(y2, y0, y0, op=OP.mult)
                vy2 = epi.tile([1, 128], mybir.dt.float32,
                               name=f"vy{ci}", tag="vy")[:, 0:nloc]
                nc.vector.tensor_tensor(vy2, y2, veps, op=OP.mult)
                h = epi.tile([1, 128], mybir.dt.float32,
                             name=f"h{ci}", tag="h")[:, 0:nloc]
                nc.vector.scalar_tensor_tensor(
                    out=h, in0=vy2, scalar=-0.5, in1=c15row[:, 0:nloc],
                    op0=OP.mult, op1=OP.add)
                rs = epi.tile([1, 128], mybir.dt.bfloat16, name=f"rs{ci}", tag="rs")
                rs = rs[:, 0:nloc]
                nc.vector.tensor_tensor(rs, y0, h, op=OP.mult)
                nc.tensor.matmul(misc[:, 0:nloc], row1, rs[:, :],
                                 start=True, stop=True)
                # PSUM->SBUF copy (ACT, tableless) so the final DVE join has
                # a single PSUM operand; hides under the FWHT matmul.
                rsbs = epi.tile([128, 128], mybir.dt.float32,
                                name=f"rb{ci}", tag="rb")
                rsbs = rsbs[:, 0:nloc]
                nc.scalar.copy(out=rsbs, in_=misc[:, 0:nloc])

                # branch rot: rot = P @ (comp*sdup') + comp*cdup'  (norm_w
                # folded into the tables; RMS scale applied after the FWHT)
                t1 = epi.tile([128, 128], mybir.dt.bfloat16, name=f"t1{ci}", tag="t1")
                t1 = t1[:, 0:nloc]
                nc.vector.tensor_tensor(t1, comp, sdup[:, b0:b0 + nloc], op=OP.mult)
                t2 = epi.tile([128, 128], mybir.dt.float32, name=f"t2{ci}", tag="t2")
                t2 = t2[:, 0:nloc]
                nc.vector.tensor_tensor(t2, comp, cdup[:, b0:b0 + nloc], op=OP.mult)
                nc.tensor.matmul(misc[:, 128:128 + nloc], pmat, t1[:, :],
                                 start=True, stop=True)
                rot = epi.tile([128, 128], mybir.dt.bfloat16, name=f"rt{ci}", tag="rt")
                rot = rot[:, 0:nloc]
                nc.vector.tensor_tensor(rot, misc[:, 128:128 + nloc], t2, op=OP.add)
                nc.tensor.matmul(misc[:, 256:256 + nloc], hmat, rot[:, :],
                                 start=True, stop=True)

                # join: out = fwht * rs_broadcast, straight into SBUF
                nc.vector.tensor_tensor(outsb[:, b0:b0 + nloc],
                                        misc[:, 256:256 + nloc],
                                        rsbs, op=OP.mult)
                nc.scalar.dma_start(out=out_d[:, b0:b0 + nloc],
                                    in_=outsb[:, b0:b0 + nloc])

            for ci, qt in enumerate(CHUNKS):
                cols = qt + RATIO
                if ci == 0:
                    # unpacked: one full bank per m-group
                    kv1 = ps.tile([128, 512], mybir.dt.float32,
                                  name="c0kv1", tag="bankA")
                    sc1 = ps.tile([128, 512], mybir.dt.float32,
                                  name="c0sc1", tag="bankA")
                    kv2 = ps.tile([128, 512], mybir.dt.float32,
                                  name="c0kv2", tag="bankB")
                    sc2 = ps.tile([128, 512], mybir.dt.float32,
                                  name="c0sc2", tag="bankB")
                    psums = (kv1, sc1, kv2, sc2)
                    chunk_matmuls(ci, psums, packed=False)
                else:
                    if ci == 1:
                        # gap fillers: keep the PE busy/warm while chunk 0's
                        # epilogue head releases the PSUM slots
                        for i in range(NBDUMMY):
                            nc.tensor.matmul(miscs[2][:, :], zt[:, 0:128],
                                             zt[:, :], start=True, stop=True)
                    bankA = ps.tile([128, 512], mybir.dt.float32,
                                    name=f"bankA{ci}", tag="bankA")
                    bankB = ps.tile([128, 512], mybir.dt.float32,
                                    name=f"bankB{ci}", tag="bankB")
                    psums = (bankA[:, 0:cols], bankA[:, cols:2 * cols],
                             bankB[:, 0:cols], bankB[:, cols:2 * cols])
                    chunk_matmuls(ci, psums, packed=True)
                epilogue(ci, psums)

    nc.finalize()
    return nc


def _prep_inputs(x, ape, wkv_w, wgate_w, norm_w, cos, sin):
    """Host-side packing of per-core input maps."""
    x = np.asarray(x, dtype=F32)[0]          # [SEQ, DIM]
    ape = np.asarray(ape, dtype=F32)         # [RATIO, 256]
    wkv_w = np.asarray(wkv_w, dtype=F32)     # [256, DIM]
    wgate_w = np.asarray(wgate_w, dtype=F32)
    norm_w = np.asarray(norm_w, dtype=F32)   # [HD]
    cos = np.asarray(cos, dtype=F32)         # [NB, 32]
    sin = np.asarray(sin, dtype=F32)

    xb = x.astype(BF16)

    w_comb = np.concatenate(
        [wkv_w[0:128], wgate_w[0:128], wkv_w[128:256], wgate_w[128:256]], axis=0
    )  # [512, DIM]
    wp = (
        w_comb.T.reshape(NG, G, 128, 512)
        .transpose(0, 2, 1, 3)
        .reshape(NG * 128, G * 512)
        .astype(BF16)
    )
    wp = np.ascontiguousarray(wp)

    pmat = np.zeros((128, 128), dtype=F32)
    for i in range(32):
        pmat[2 * i, 2 * i + 1] = 1.0
        pmat[2 * i + 1, 2 * i] = 1.0
    hmat = _fwht_mat()

    cbk = np.zeros((128, B_TOT), dtype=F32)
    cbk[:, B_PM:B_PM + 128] = pmat
    cbk[:, B_HM:B_HM + 128] = hmat
    cbk[0, B_R1:B_R1 + 128] = 1.0
    cbk[:, B_OK] = 1.0 / HD
    cbk = np.ascontiguousarray(cbk.astype(BF16))

    # per-phase tiled ape consts [128, 512]: ape*[c, j] = ape[j % 4, c(+128)]
    apeL = np.tile(ape[:, 0:128].T, (1, 128))     # [128, 512]
    apeH = np.tile(ape[:, 128:256].T, (1, 128))

    in_maps = []
    for c in range(NCORES):
        t0c = c * TOK

        def pack_chunk(ci):
            qt, o = CHUNKS[ci], OFFS[ci]
            g0 = t0c + o
            # columns: [4 halo tokens | qt own tokens]
            block = np.zeros((qt + RATIO, DIM), dtype=BF16)
            if g0 >= RATIO:
                block[0:RATIO] = xb[g0 - RATIO:g0]
            block[RATIO:] = xb[g0:g0 + qt]
            segT = np.ascontiguousarray(block.T)      # [DIM, cols]
            cols = qt + RATIO
            a = segT.reshape(NG, G, 128, cols).transpose(0, 2, 1, 3)
            return a.reshape(NG * 128, G * cols)

        xpA = np.ascontiguousarray(pack_chunk(0))
        xpB = np.ascontiguousarray(
            np.concatenate([pack_chunk(1), pack_chunk(2)], axis=0))
        xpC = np.ascontiguousarray(pack_chunk(3))

        b0 = c * NBC
        cs = cos[b0:b0 + NBC]                       # [NBC, 32]
        ss = sin[b0:b0 + NBC]
        cpk = np.zeros((128, C_TOT), dtype=F32)
        cd = np.ones((128, NBC), dtype=F32)
        sd = np.zeros((128, NBC), dtype=F32)
        cd[0:64:2] = cs.T
        cd[1:64:2] = cs.T
        # pair-permuted sin table: the sin multiply happens before the
        # pair-swap matmul, so sdupP[c] = sigma(c^1) * sin
        sd[0:64:2] = ss.T
        sd[1:64:2] = -ss.T
        # fold norm_w into both tables (RMS rs scale applied post-FWHT)
        cd *= norm_w[:, None]
        sd *= norm_w[:, None]
        cpk[:, C_CD:C_CD + NBC] = cd
        cpk[:, C_SD:C_SD + NBC] = sd
        cpk[:, C_AL:C_AL + 512] = apeL
        cpk[:, C_AH:C_AH + 512] = apeH
        cpk[:, C_HB] = NEGB if c == 0 else 0.0
        cpk[0, C_EP:C_EP + 128] = EPS
        cpk[0, C_MG:C_MG + 128] = np.full(
            128, 0x5F3759DF, dtype=np.uint32).view(np.float32)
        cpk[0, C_ON:C_ON + 128] = np.full(
            128, 1, dtype=np.uint32).view(np.float32)
        cpk[0, C_15:C_15 + 128] = 1.5

        in_maps.append(dict(xpA=xpA, xpB=xpB, xpC=xpC, wp=wp,
                            cpk=np.ascontiguousarray(cpk), cbk=cbk))
    return in_maps


LAST_RESULTS = None


def kernel(x, ape, wkv_w, wgate_w, norm_w, cos, sin, start_pos=0,
           compress_state=None, **_unused):
    global LAST_RESULTS
    in_maps = _prep_inputs(x, ape, wkv_w, wgate_w, norm_w, cos, sin)
    if "nc" not in _cache:
        _cache["nc"] = _build_nc()
    nc = _cache["nc"]
    trace = bool(int(os.environ.get("KERNEL_TRACE", "0") or 0))
    tdir = os.environ.get("KERNEL_TRACE_DIR") or None
    res = run_bass_kernel_spmd(
        nc, in_maps, core_ids=list(range(NCORES)),
        trace=trace,
        trace_cores=[0] if trace else None,
        tmpdir=tdir,
    )
    LAST_RESULTS = res
    out = np.empty((1, NB, HD), dtype=F32)
    for c in range(NCORES):
        out[0, c * NBC:(c + 1) * NBC, :] = res.results[c]["out"].T
    return out
